# revision 1
# baseline (speedup 1.0000x reference)
"""Trainium2 Bass kernel for the per-feature MLP ensemble (dense_mlp).

Reference computation (per feature f of F=128 independent tiny MLPs):
    h1 = elu(X @ W1[f] + b1[f])        X:[N,160]  W1[f]:[160,32]
    h2 = elu(h1 @ W2[f] + b2[f])       W2[f]:[32,32]
    out[:, f] = h2 @ W3[f] + b3[f]     W3[f]:[32]

Strategy:
  - Data-parallel: shard N=32768 rows across 8 cores (4096 each),
    replicate the (tiny) weights.
  - Transposed layout on chip: channels (f,h) on SBUF partitions, n on
    the free dimension.  The F networks are processed in 32 groups of 4
    features = 128 channels, so layer 2 is a single 128x128 block-diagonal
    matmul per group and layer 3 a 128x4 matmul per group.
  - ELU via the exact identity   elu(y) + 1 = max(y + 1, min(e^y, 1))
    (valid because e^y >= 1 + y).  With psum holding y+1 (bias row folded
    into the layer-1 matmul), each ELU costs one ScalarE Exp and one
    VectorE fused scalar_tensor_tensor op:
        u = (e  min  1.0)  max  psum        # u = elu(y)+1
    The +1 offsets are linear, so they fold into the next layer's bias.
"""

import numpy as np

import concourse.bass as bass
import concourse.bacc as bacc
import concourse.mybir as mybir
import concourse.tile as tile
from concourse.bass_utils import run_bass_kernel_spmd

N, D, F, H = 32768, 160, 128, 32
NCORES = 8
NS = N // NCORES          # rows per core
CH = F * H                # 4096 channels after layer 1
GROUPS = F // 4           # 32 groups of 4 features (=128 channels)
CHUNK = 512               # free-dim (n) tile size
NCHUNKS = NS // CHUNK

FP16 = mybir.dt.float16
F32 = mybir.dt.float32
AF = mybir.ActivationFunctionType
ALU = mybir.AluOpType


def _build_bass():
    nc = bacc.Bacc("TRN2", target_bir_lowering=False, debug=False,
                   num_devices=NCORES)

    def inp(name, shape, dt):
        return nc.dram_tensor(name, shape, dt, kind="ExternalInput").ap()

    xt_a = inp("xt_a", [128, NS], FP16)        # X.T rows 0..127 (shard)
    xt_b = inp("xt_b", [33, NS], FP16)         # X.T rows 128..159 + ones row
    w1a = inp("w1a", [128, CH], FP16)          # W1' rows 0..127
    w1b = inp("w1b", [33, CH], FP16)           # W1' rows 128..159 + (b1+1) row
    w2b = inp("w2b", [128, GROUPS * 128], FP16)  # blockdiag(W2) per group
    w3b = inp("w3b", [128, GROUPS * 128], FP16)  # W3 cols placed at out partition
    c2 = inp("c2", [128, GROUPS], F32)         # b2 - colsum(W2), per channel
    c2p1 = inp("c2p1", [128, GROUPS], F32)     # c2 + 1
    b3pp = inp("b3pp", [128, 1], F32)          # b3 + W3 . c2
    neg1 = inp("neg1", [128, 1], F32)          # constant -1 bias column
    outT = nc.dram_tensor("outT", [128, NS], F32, kind="ExternalOutput").ap()

    from contextlib import ExitStack
    with tile.TileContext(nc) as tc, ExitStack() as ctx:
        wp = ctx.enter_context(tc.tile_pool(name="w", bufs=1))

        def load(ap_dram, shape, dt, tag):
            t = wp.tile(list(shape), dt, tag=tag)
            nc.sync.dma_start(t[:], ap_dram)
            return t

        xt_a_sb = load(xt_a, [128, NS], FP16, "xt_a")
        xt_b_sb = load(xt_b, [33, NS], FP16, "xt_b")
        w1a_sb = load(w1a, [128, CH], FP16, "w1a")
        w1b_sb = load(w1b, [33, CH], FP16, "w1b")
        w2b_sb = load(w2b, [128, GROUPS * 128], FP16, "w2b")
        w3b_sb = load(w3b, [128, GROUPS * 128], FP16, "w3b")
        c2_sb = load(c2, [128, GROUPS], F32, "c2")
        c2p1_sb = load(c2p1, [128, GROUPS], F32, "c2p1")
        b3_sb = load(b3pp, [128, 1], F32, "b3pp")
        neg1_sb = load(neg1, [128, 1], F32, "neg1")

        # Warm the ACT Exp table on a tiny tile so the table-load pseudo-op
        # walrus inserts before the first Exp lands on a low-dependency
        # instruction instead of the first real activation.
        warm = wp.tile([128, 1], FP16, tag="warm")
        nc.scalar.activation(warm[:], neg1_sb[:], AF.Exp,
                             bias=neg1_sb[:, 0:1])

        ip = ctx.enter_context(tc.tile_pool(name="interm", bufs=3))
        p1 = ctx.enter_context(tc.tile_pool(name="p1", bufs=2, space="PSUM"))
        p2 = ctx.enter_context(tc.tile_pool(name="p2", bufs=2, space="PSUM"))
        po = ctx.enter_context(tc.tile_pool(name="po", bufs=2, space="PSUM"))
        op = ctx.enter_context(tc.tile_pool(name="osb", bufs=2))

        for ci in range(NCHUNKS):
            cs = slice(ci * CHUNK, (ci + 1) * CHUNK)
            pout = po.tile([128, CHUNK], F32)
            for g in range(GROUPS):
                g128 = slice(128 * g, 128 * (g + 1))
                # ---- layer 1: psum1 = z1 + 1 (bias+1 baked into ones row)
                ps1 = p1.tile([128, CHUNK], F32)
                nc.tensor.matmul(ps1[:], w1a_sb[:, g128], xt_a_sb[:, cs],
                                 start=True, stop=False)
                nc.tensor.matmul(ps1[:], w1b_sb[:, g128], xt_b_sb[:, cs],
                                 start=False, stop=True)
                e1 = ip.tile([128, CHUNK], FP16, tag="e1")
                nc.scalar.activation(e1[:], ps1[:], AF.Exp,
                                     bias=neg1_sb[:, 0:1])
                u1 = ip.tile([128, CHUNK], FP16, tag="u1")
                nc.vector.scalar_tensor_tensor(
                    u1[:], e1[:], 1.0, ps1[:], ALU.min, ALU.max)
                # ---- layer 2: psum2 = z2 - c2
                ps2 = p2.tile([128, CHUNK], F32)
                nc.tensor.matmul(ps2[:], w2b_sb[:, g128], u1[:],
                                 start=True, stop=True)
                e2 = ip.tile([128, CHUNK], FP16, tag="e2")
                nc.scalar.activation(e2[:], ps2[:], AF.Exp,
                                     bias=c2_sb[:, g:g + 1])
                v2 = ip.tile([128, CHUNK], FP16, tag="v2")
                nc.gpsimd.tensor_scalar(
                    v2[:], e2[:], 1.0, c2p1_sb[:, g:g + 1],
                    ALU.min, ALU.subtract)
                u2 = ip.tile([128, CHUNK], FP16, tag="u2")
                nc.vector.scalar_tensor_tensor(
                    u2[:], v2[:], 0.0, ps2[:], ALU.add, ALU.max)
                # ---- layer 3: accumulate all groups into one [128,CHUNK]
                # psum tile; group g's lhsT has its 4 features' W3 in
                # columns 4g..4g+3, zeros elsewhere.
                nc.tensor.matmul(pout[:], w3b_sb[:, g128], u2[:],
                                 start=(g == 0), stop=(g == GROUPS - 1))
            osb = op.tile([128, CHUNK], F32)
            nc.vector.tensor_scalar(osb[:], pout[:], b3_sb[:, 0:1], None,
                                    ALU.add)
            nc.sync.dma_start(outT[:, cs], osb[:])
    nc.compile()
    return nc


def _prep_inputs(X, W1, b1, W2, b2, W3, b3):
    X = np.asarray(X, np.float32)
    W1 = np.asarray(W1, np.float32)
    b1 = np.asarray(b1, np.float32)
    W2 = np.asarray(W2, np.float32)
    b2 = np.asarray(b2, np.float32)
    W3 = np.asarray(W3, np.float32)
    b3 = np.asarray(b3, np.float32)

    W1p = W1.transpose(1, 0, 2).reshape(D, CH)
    b1p = b1.reshape(CH)
    w1a = np.ascontiguousarray(W1p[0:128]).astype(np.float16)
    w1b = np.concatenate([W1p[128:160], (b1p + 1.0)[None, :]], 0).astype(np.float16)

    XT = X.T
    xt_a_full = np.ascontiguousarray(XT[0:128]).astype(np.float16)
    xt_b_full = np.concatenate(
        [XT[128:160], np.ones((1, N), np.float32)], 0).astype(np.float16)

    w2blk = np.zeros((128, GROUPS * 128), np.float32)
    for g in range(GROUPS):
        for j in range(4):
            f = 4 * g + j
            w2blk[32 * j:32 * (j + 1),
                  128 * g + 32 * j:128 * g + 32 * (j + 1)] = W2[f]
    w2blk = w2blk.astype(np.float16)

    colsum2 = W2.sum(axis=1)                       # [F, H]
    c2_ch = (b2 - colsum2).reshape(CH)
    c2 = np.ascontiguousarray(c2_ch.reshape(GROUPS, 128).T).astype(np.float32)
    c2p1 = (c2 + 1.0).astype(np.float32)

    w3blk = np.zeros((128, GROUPS * 128), np.float32)
    for g in range(GROUPS):
        for j in range(4):
            f = 4 * g + j
            w3blk[32 * j:32 * (j + 1), 128 * g + f] = W3[f]
    w3blk = w3blk.astype(np.float16)

    b3pp = (b3 + (W3 * c2_ch.reshape(F, H)).sum(1)).astype(np.float32)
    b3pp = b3pp.reshape(128, 1)
    neg1 = np.full((128, 1), -1.0, np.float32)

    shared = dict(w1a=w1a, w1b=w1b, w2b=w2blk, w3b=w3blk,
                  c2=c2, c2p1=c2p1, b3pp=b3pp, neg1=neg1)
    in_maps = []
    for c in range(NCORES):
        sl = slice(c * NS, (c + 1) * NS)
        m = dict(shared)
        m["xt_a"] = np.ascontiguousarray(xt_a_full[:, sl])
        m["xt_b"] = np.ascontiguousarray(xt_b_full[:, sl])
        in_maps.append(m)
    return in_maps


_NC_CACHE = {}


def _get_nc():
    if "nc" not in _NC_CACHE:
        _NC_CACHE["nc"] = _build_bass()
    return _NC_CACHE["nc"]


def kernel(X, W1, b1, W2, b2, W3, b3, trace=False, trace_kwargs=None):
    nc = _get_nc()
    in_maps = _prep_inputs(X, W1, b1, W2, b2, W3, b3)
    res = run_bass_kernel_spmd(nc, in_maps, list(range(NCORES)),
                               trace=trace, **(trace_kwargs or {}))
    outs = res.results
    outT = np.concatenate([outs[c]["outT"] for c in range(NCORES)], axis=1)
    out = np.ascontiguousarray(outT.T).astype(np.float32)
    if trace:
        kernel.last_results = res
    return out



# revision 3
# speedup vs baseline: 2.8314x; 2.8314x over previous
"""Trainium2 Bass kernel for the per-feature MLP ensemble (dense_mlp).

Reference computation (per feature f of F=128 independent tiny MLPs):
    h1 = elu(X @ W1[f] + b1[f])        X:[N,160]  W1[f]:[160,32]
    h2 = elu(h1 @ W2[f] + b2[f])       W2[f]:[32,32]
    out[:, f] = h2 @ W3[f] + b3[f]     W3[f]:[32]

Strategy:
  - Data-parallel: shard N=32768 rows across 8 cores (4096 each),
    replicate the (tiny) weights.
  - Transposed layout on chip: channels (f,h) on SBUF partitions, n on
    the free dimension.  The F networks are processed in 32 groups of 4
    features = 128 channels, so layer 2 is a single 128x128 block-diagonal
    matmul per group and layer 3 a 128x4 matmul per group.
  - ELU via the exact identity   elu(y) + 1 = max(y + 1, min(e^y, 1))
    (valid because e^y >= 1 + y).  Both layers keep psum = y + 1: layer 1
    folds b1+1 into a ones row of the K=161 matmul; layer 2 accumulates a
    rank-1 (K=1) ones-row matmul carrying (b2 - colsum(W2) + 1) into its
    psum.  Each ELU then costs one ScalarE Exp (bias -1) and one VectorE
    fused scalar_tensor_tensor:  u = (e min 1.0) max psum  =  elu(y)+1.
    The +1 offsets are linear and fold into the next layer's bias.
    (The previous revision used a Pool-engine tensor_scalar here; on HW
    that op ran at ~7.5us per [128,512] tile and serialized the kernel.)
  - Software pipelining: the PE instruction queue is in-order, so each
    step issues layer 1 of group t, layer 2 of group t-1 and layer 3 of
    group t-2, keeping the PE busy while ACT/DVE produce the activations
    the next PE instruction needs.
"""

import numpy as np

import concourse.bass as bass
import concourse.bacc as bacc
import concourse.mybir as mybir
import concourse.tile as tile
from concourse.bass_utils import run_bass_kernel_spmd

N, D, F, H = 32768, 160, 128, 32
NCORES = 8
NS = N // NCORES          # rows per core
CH = F * H                # 4096 channels after layer 1
GROUPS = F // 4           # 32 groups of 4 features (=128 channels)
CHUNK = 512               # free-dim (n) tile size
NCHUNKS = NS // CHUNK
TOTAL = NCHUNKS * GROUPS  # flat pipeline steps

FP16 = mybir.dt.float16
F32 = mybir.dt.float32
AF = mybir.ActivationFunctionType
ALU = mybir.AluOpType


def _build_bass():
    nc = bacc.Bacc("TRN2", target_bir_lowering=False, debug=False,
                   num_devices=NCORES)

    def inp(name, shape, dt):
        return nc.dram_tensor(name, shape, dt, kind="ExternalInput").ap()

    xt_a = inp("xt_a", [128, NS], FP16)        # X.T rows 0..127 (shard)
    xt_b = inp("xt_b", [33, NS], FP16)         # X.T rows 128..159 + ones row
    w1a = inp("w1a", [128, CH], FP16)          # W1' rows 0..127
    w1b = inp("w1b", [33, CH], FP16)           # W1' rows 128..159 + (b1+1) row
    w2b = inp("w2b", [128, GROUPS * 128], FP16)  # blockdiag(W2) per group
    w3b = inp("w3b", [128, GROUPS * 128], FP16)  # W3 cols placed at out partition
    c2p1 = inp("c2p1", [1, GROUPS * 128], FP16)  # b2 - colsum(W2) + 1, row layout
    ones = inp("ones", [1, NS], FP16)          # ones row for the K=1 bias matmul
    b3pp = inp("b3pp", [128, 1], F32)          # b3 - colsum(W3)
    neg1 = inp("neg1", [128, 1], F32)          # constant -1 bias column
    outT = nc.dram_tensor("outT", [128, NS], F32, kind="ExternalOutput").ap()

    from contextlib import ExitStack
    with tile.TileContext(nc) as tc, ExitStack() as ctx:
        wp = ctx.enter_context(tc.tile_pool(name="w", bufs=1))

        def load(ap_dram, shape, dt, tag):
            t = wp.tile(list(shape), dt, tag=tag)
            nc.sync.dma_start(t[:], ap_dram)
            return t

        xt_a_sb = load(xt_a, [128, NS], FP16, "xt_a")
        xt_b_sb = load(xt_b, [33, NS], FP16, "xt_b")
        w1a_sb = load(w1a, [128, CH], FP16, "w1a")
        w1b_sb = load(w1b, [33, CH], FP16, "w1b")
        w2b_sb = load(w2b, [128, GROUPS * 128], FP16, "w2b")
        w3b_sb = load(w3b, [128, GROUPS * 128], FP16, "w3b")
        c2p1_sb = load(c2p1, [1, GROUPS * 128], FP16, "c2p1")
        ones_sb = load(ones, [1, NS], FP16, "ones")
        b3_sb = load(b3pp, [128, 1], F32, "b3pp")
        neg1_sb = load(neg1, [128, 1], F32, "neg1")

        # Warm the ACT Exp table on a tiny tile so the table-load pseudo-op
        # lands on a low-dependency instruction instead of the first real
        # activation.
        warm = wp.tile([128, 1], FP16, tag="warm")
        nc.scalar.activation(warm[:], neg1_sb[:], AF.Exp,
                             bias=neg1_sb[:, 0:1])

        ip = ctx.enter_context(tc.tile_pool(name="interm", bufs=3))
        p1 = ctx.enter_context(tc.tile_pool(name="p1", bufs=3, space="PSUM"))
        p2 = ctx.enter_context(tc.tile_pool(name="p2", bufs=3, space="PSUM"))
        po = ctx.enter_context(tc.tile_pool(name="po", bufs=2, space="PSUM"))
        op = ctx.enter_context(tc.tile_pool(name="osb", bufs=2))

        ps1_t, ps2_t, u1_t, u2_t, pout_t = {}, {}, {}, {}, {}

        def cslice(t):
            ci = t // GROUPS
            return slice(ci * CHUNK, (ci + 1) * CHUNK)

        def gslice(t):
            g = t % GROUPS
            return slice(128 * g, 128 * (g + 1))

        def front(t):
            # PE: layer-1 matmuls + layer-2 bias (rank-1) matmul of step t
            cs, g128 = cslice(t), gslice(t)
            ps2 = p2.tile([128, CHUNK], F32, tag="ps2")
            nc.tensor.matmul(ps2[:], c2p1_sb[0:1, g128], ones_sb[0:1, cs],
                             start=True, stop=False)
            ps1 = p1.tile([128, CHUNK], F32, tag="ps1")
            nc.tensor.matmul(ps1[:], w1a_sb[:, g128], xt_a_sb[:, cs],
                             start=True, stop=False)
            nc.tensor.matmul(ps1[:], w1b_sb[:, g128], xt_b_sb[:, cs],
                             start=False, stop=True)
            ps1_t[t], ps2_t[t] = ps1, ps2

        def mid(t):
            # ACT+DVE: layer-1 ELU of step t; PE: layer-2 matmul of step t
            ps1, ps2 = ps1_t.pop(t), ps2_t[t]
            e1 = ip.tile([128, CHUNK], FP16, tag="e1")
            nc.scalar.activation(e1[:], ps1[:], AF.Exp,
                                 bias=neg1_sb[:, 0:1])
            u1 = ip.tile([128, CHUNK], FP16, tag="u1")
            nc.vector.scalar_tensor_tensor(
                u1[:], e1[:], 1.0, ps1[:], ALU.min, ALU.max)
            nc.tensor.matmul(ps2[:], w2b_sb[:, gslice(t)], u1[:],
                             start=False, stop=True)

        def back(t):
            # ACT+DVE: layer-2 ELU of step t; PE: layer-3 matmul of step t
            ps2 = ps2_t.pop(t)
            g = t % GROUPS
            e2 = ip.tile([128, CHUNK], FP16, tag="e2")
            nc.scalar.activation(e2[:], ps2[:], AF.Exp,
                                 bias=neg1_sb[:, 0:1])
            u2 = ip.tile([128, CHUNK], FP16, tag="u2")
            nc.vector.scalar_tensor_tensor(
                u2[:], e2[:], 1.0, ps2[:], ALU.min, ALU.max)
            if g == 0:
                pout_t[t // GROUPS] = po.tile([128, CHUNK], F32, tag="pout",
                                              name="pout")
            pout = pout_t[t // GROUPS]
            nc.tensor.matmul(pout[:], w3b_sb[:, gslice(t)], u2[:],
                             start=(g == 0), stop=(g == GROUPS - 1))
            if g == GROUPS - 1:
                ci = t // GROUPS
                osb = op.tile([128, CHUNK], F32, tag="osb")
                nc.vector.tensor_scalar(osb[:], pout[:], b3_sb[:, 0:1], None,
                                        ALU.add)
                del pout_t[ci]
                nc.sync.dma_start(outT[:, cslice(t)], osb[:])

        for t in range(TOTAL + 2):
            if t < TOTAL:
                front(t)
            if 0 <= t - 1 < TOTAL:
                mid(t - 1)
            if 0 <= t - 2 < TOTAL:
                back(t - 2)
    nc.compile()
    return nc


def _prep_inputs(X, W1, b1, W2, b2, W3, b3):
    X = np.asarray(X, np.float32)
    W1 = np.asarray(W1, np.float32)
    b1 = np.asarray(b1, np.float32)
    W2 = np.asarray(W2, np.float32)
    b2 = np.asarray(b2, np.float32)
    W3 = np.asarray(W3, np.float32)
    b3 = np.asarray(b3, np.float32)

    W1p = W1.transpose(1, 0, 2).reshape(D, CH)
    b1p = b1.reshape(CH)
    w1a = np.ascontiguousarray(W1p[0:128]).astype(np.float16)
    w1b = np.concatenate([W1p[128:160], (b1p + 1.0)[None, :]], 0).astype(np.float16)

    XT = X.T
    xt_a_full = np.ascontiguousarray(XT[0:128]).astype(np.float16)
    xt_b_full = np.concatenate(
        [XT[128:160], np.ones((1, N), np.float32)], 0).astype(np.float16)

    w2blk = np.zeros((128, GROUPS * 128), np.float32)
    for g in range(GROUPS):
        for j in range(4):
            f = 4 * g + j
            w2blk[32 * j:32 * (j + 1),
                  128 * g + 32 * j:128 * g + 32 * (j + 1)] = W2[f]
    w2blk = w2blk.astype(np.float16)

    colsum2 = W2.sum(axis=1)                       # [F, H]
    c2p1 = (b2 - colsum2 + 1.0).reshape(1, CH).astype(np.float16)

    w3blk = np.zeros((128, GROUPS * 128), np.float32)
    for g in range(GROUPS):
        for j in range(4):
            f = 4 * g + j
            w3blk[32 * j:32 * (j + 1), 128 * g + f] = W3[f]
    w3blk = w3blk.astype(np.float16)

    b3pp = (b3 - W3.sum(axis=1)).astype(np.float32).reshape(128, 1)
    neg1 = np.full((128, 1), -1.0, np.float32)
    ones_row = np.ones((1, N), np.float16)

    shared = dict(w1a=w1a, w1b=w1b, w2b=w2blk, w3b=w3blk,
                  c2p1=c2p1, b3pp=b3pp, neg1=neg1)
    in_maps = []
    for c in range(NCORES):
        sl = slice(c * NS, (c + 1) * NS)
        m = dict(shared)
        m["xt_a"] = np.ascontiguousarray(xt_a_full[:, sl])
        m["xt_b"] = np.ascontiguousarray(xt_b_full[:, sl])
        m["ones"] = np.ascontiguousarray(ones_row[:, sl])
        in_maps.append(m)
    return in_maps


_NC_CACHE = {}


def _get_nc():
    if "nc" not in _NC_CACHE:
        _NC_CACHE["nc"] = _build_bass()
    return _NC_CACHE["nc"]


def kernel(X, W1, b1, W2, b2, W3, b3, trace=False, trace_kwargs=None):
    nc = _get_nc()
    in_maps = _prep_inputs(X, W1, b1, W2, b2, W3, b3)
    res = run_bass_kernel_spmd(nc, in_maps, list(range(NCORES)),
                               trace=trace, **(trace_kwargs or {}))
    outs = res.results
    outT = np.concatenate([outs[c]["outT"] for c in range(NCORES)], axis=1)
    out = np.ascontiguousarray(outT.T).astype(np.float32)
    if trace:
        kernel.last_results = res
    return out
